# revision 17
# baseline (speedup 1.0000x reference)
"""GPT self-attention layer (B=2, S=2048, D=1024, H=16, hd=64) on 8 TRN2 cores.

Sharding: data-parallel over batch (2) x tensor-parallel over heads (4 groups
of 4 heads). Core c handles batch b=c//4, head group g=c%4.

Per-core pipeline (all matmuls in float32r, ~1.5e-4 relative rounding):
  1. Transpose x [2048,1024] -> xT [1024-part, 2048] via PE transposes.
  2. QT/KT = W.T @ x.T (+bias via ACT copy), V = x @ Wv (+bias via DVE add),
     V stored with an interleaved ones-column per head (softmax denominator).
  3. Attention per head, scoresT orientation [k-part, q-free]:
     scoresT = KT.T @ QT (row-tiled 64x128, two heads concurrently),
     pT = exp(0.125 * scoresT) on ACT, causal diag masked by DVE multiply,
     psum_c[65,512] += [V|1].T @ pT accumulated over k-chunks
     (row 64 = softmax denominator), then normalize:
     recip (DVE) -> K=1 broadcast matmul at partition base 64 -> DVE mult.
  4. AllToAll (8 cores, shards duplicated across batch halves so the program
     is core-independent); receive side uses dma_gather with per-core indices.
  5. out = ctxT_full.T @ Wo + bo (bo added via DVE using a partition-broadcast
     copy of bo), DMA to y [512, 1024] (this core's token block).

build(stage): stage in {"proj", "attn", "a2a", "full"} for bisection; partial
stages write debug data to y instead of the final output.
"""

import contextlib
import ctypes
import sys
import types

sys.path.insert(0, "/opt/trn_rl_repo")

import numpy as np

import concourse.bass as bass
import concourse.mybir as mybir
import concourse.tile as tile
from concourse import bacc
from concourse import bass_utils
from concourse.masks import make_identity

P = 128
B, S, D = 2, 2048, 1024
NH_LOC = 4          # heads per core
HD = 64             # head dim
G = NH_LOC * HD     # local head dims = 256
MC = G // P         # m-chunks of local dims = 2
DC = D // P         # d-chunks = 8
TB = 512            # token block (output tokens per core, q-tile width)
NQT = S // TB       # q-tiles = 4
NTC = S // P        # token chunks = 16
NC = 8

F32 = mybir.dt.float32
F32R = mybir.dt.float32r
I16 = mybir.dt.int16
Exp = mybir.ActivationFunctionType.Exp
Ident = mybir.ActivationFunctionType.Identity
MULT = mybir.AluOpType.mult
ADD = mybir.AluOpType.add

_STAGES = {"proj": 1, "attn": 2, "a2a": 3, "full": 4}


def _install_ntff_hook():
    """Make trace=True work under axon: inject antenv.axon_hooks backed by
    ctypes calls into libaxon_pjrt.so (mirrors trn_agent_boot logic)."""
    if "antenv.axon_hooks" in sys.modules:
        return
    holder = {}
    mod = types.ModuleType("antenv.axon_hooks")
    mod.set_axon_ntff_profile_hook = lambda h: holder.update(h=h)
    mod.get_axon_ntff_profile_hook = lambda: holder.get("h")
    sys.modules["antenv.axon_hooks"] = mod
    try:
        lib = ctypes.CDLL("/opt/axon/libaxon_pjrt.so")
        if not hasattr(lib, "axon_start_nrt_profile"):
            return
    except OSError:
        return
    lib.axon_start_nrt_profile.argtypes = [
        ctypes.POINTER(ctypes.c_int64),
        ctypes.c_size_t,
    ]
    lib.axon_start_nrt_profile.restype = ctypes.c_int64
    lib.axon_stop_nrt_profile.argtypes = [ctypes.c_char_p]
    lib.axon_stop_nrt_profile.restype = ctypes.c_int64

    @contextlib.contextmanager
    def _hook(output_dir, device_ids):
        import jax

        jax.devices()
        if device_ids:
            ids = (ctypes.c_int64 * len(device_ids))(*device_ids)
            rc = lib.axon_start_nrt_profile(ids, len(device_ids))
        else:
            rc = lib.axon_start_nrt_profile(None, 0)
        if rc != 0:
            raise RuntimeError(f"axon_start_nrt_profile rc={rc}")
        try:
            yield
        finally:
            n = lib.axon_stop_nrt_profile(str(output_dir).encode())
            print(f"profile: {n} ntff file(s) written to {output_dir}")

    holder["h"] = _hook


def build(stage="full", coll=True, gather=True):
    st = _STAGES[stage]
    nc = bacc.Bacc("TRN2", target_bir_lowering=False, debug=False, num_devices=NC)

    x_d = nc.dram_tensor("x", [S, D], F32, kind="ExternalInput").ap()
    wq_d = nc.dram_tensor("wq", [D, G], F32R, kind="ExternalInput").ap()
    wk_d = nc.dram_tensor("wk", [D, G], F32R, kind="ExternalInput").ap()
    wv_d = nc.dram_tensor("wv", [D, G], F32R, kind="ExternalInput").ap()
    bq_d = nc.dram_tensor("bq", [P, MC], F32, kind="ExternalInput").ap()
    bk_d = nc.dram_tensor("bk", [P, MC], F32, kind="ExternalInput").ap()
    bv_d = nc.dram_tensor("bv", [1, G], F32, kind="ExternalInput").ap()
    wo_d = nc.dram_tensor("wo", [D, D], F32R, kind="ExternalInput").ap()
    bo_d = nc.dram_tensor("bo", [1, D], F32, kind="ExternalInput").ap()
    gidx_d = nc.dram_tensor("gidx", [P, D // 16], I16, kind="ExternalInput").ap()
    y_d = nc.dram_tensor("y", [TB, D], F32, kind="ExternalOutput").ap()

    with tile.TileContext(nc) as tc:
        with (
            tc.tile_pool(name="const", bufs=1) as const,
            tc.tile_pool(name="dram", bufs=1, space="DRAM") as dram,
            tc.tile_pool(name="ps_mm", bufs=4, space="PSUM") as ps_mm,
            tc.tile_pool(name="ps_ctx", bufs=2, space="PSUM") as ps_ctx,
            tc.tile_pool(name="ps_bc", bufs=2, space="PSUM") as ps_bc,
            tc.tile_pool(name="persist", bufs=1) as persist,
        ):
            # ---------------- constants ----------------
            ident = const.tile([P, P], F32, tag="ident")
            make_identity(nc, ident[:])
            ones_f = const.tile([P, 1], F32, tag="ones_f")
            nc.vector.memset(ones_f[:], 1.0)
            ones_r = const.tile([P, 65], F32R, tag="ones_r")
            nc.vector.tensor_copy(
                ones_r[:], ones_f[:, 0:1, None].to_broadcast((P, 65, 1))
            )
            # trimask[k, u] = 1 if k <= u else 0 (keep where u - k >= 0)
            tri_f = const.tile([P, P], F32, tag="tri_f")
            nc.gpsimd.memset(tri_f[:], 1.0)
            nc.gpsimd.affine_select(
                out=tri_f[:],
                in_=tri_f[:],
                compare_op=mybir.AluOpType.is_ge,
                fill=0.0,
                base=0,
                pattern=[[1, P]],
                channel_multiplier=-1,
            )
            tri_r = const.tile([P, P], F32R, tag="tri_r")
            nc.vector.tensor_copy(tri_r[:], tri_f[:])

            bq_sb = const.tile([P, MC], F32, tag="bq")
            bk_sb = const.tile([P, MC], F32, tag="bk")
            nc.sync.dma_start(bq_sb[:], bq_d)
            nc.sync.dma_start(bk_sb[:], bk_d)
            bv_row = const.tile([1, G], F32, tag="bv_row")
            nc.sync.dma_start(bv_row[:], bv_d)
            bv_bc = const.tile([P, G], F32, tag="bv_bc")
            nc.gpsimd.partition_broadcast(bv_bc[:], bv_row[:])
            bo_row = const.tile([1, D], F32, tag="bo_row")
            nc.sync.dma_start(bo_row[:], bo_d)
            bo_bc = const.tile([P, D], F32, tag="bo_bc")
            nc.gpsimd.partition_broadcast(bo_bc[:], bo_row[:])
            gidx_sb = const.tile([P, D // 16], I16, tag="gidx")
            nc.sync.dma_start(gidx_sb[:], gidx_d)

            # persistent activations
            qT = persist.tile([P, MC, S], F32R, tag="qT")
            kT = persist.tile([P, MC, S], F32R, tag="kT")
            v_sb = persist.tile([P, NTC, NH_LOC * (HD + 1)], F32R, tag="v")
            wo_sb = persist.tile([P, DC, D], F32R, tag="wo")

            # ones columns of v (denominator trick): col 64 of each head block
            v_ones_ap = v_sb[:].rearrange("p t (h c) -> p t h c", c=HD + 1)[
                :, :, :, HD
            ]
            nc.vector.tensor_copy(
                v_ones_ap,
                ones_f[:, 0:1, None].to_broadcast((P, NTC, NH_LOC, 1)),
            )

            a2a_in = dram.tile([2 * NC * P, TB], F32R)
            a2a_out = dram.tile([2 * NC * P, TB], F32R)

            with (
                tc.tile_pool(name="xw", bufs=1) as xw,
                tc.tile_pool(name="xnat", bufs=2) as xnat,
            ):
                wq_sb = xw.tile([P, DC, G], F32R, tag="wq")
                wk_sb = xw.tile([P, DC, G], F32R, tag="wk")
                wv_sb = xw.tile([P, DC, G], F32R, tag="wv")

                xT = xw.tile([P, DC, S], F32R, tag="xT")

                # ---------- phase 1: load + transpose x ----------
                for tc_i in range(NTC):
                    x_nat = xnat.tile([P, D], F32, tag="xnat")
                    nc.sync.dma_start(x_nat[:], x_d[tc_i * P : (tc_i + 1) * P, :])
                    for dcb in range(2):  # blocks of 4 d-chunks
                        tr_ps = ps_mm.tile([P, 512], F32, tag="mm")
                        for i in range(4):
                            dc = dcb * 4 + i
                            nc.tensor.transpose(
                                tr_ps[:, i * P : (i + 1) * P],
                                x_nat[:, dc * P : (dc + 1) * P],
                                ident[:],
                            )
                        nc.vector.tensor_copy(
                            xT[:, dcb * 4 : dcb * 4 + 4, tc_i * P : (tc_i + 1) * P],
                            tr_ps[:].rearrange("p (i u) -> p i u", i=4),
                        )

                nc.sync.dma_start(wq_sb[:], wq_d.rearrange("(dc p) m -> p dc m", p=P))
                nc.sync.dma_start(wk_sb[:], wk_d.rearrange("(dc p) m -> p dc m", p=P))
                nc.sync.dma_start(wv_sb[:], wv_d.rearrange("(dc p) m -> p dc m", p=P))
                # ---------- phase 2: projections ----------
                for w_sb, b_sb, out_t in ((wq_sb, bq_sb, qT), (wk_sb, bk_sb, kT)):
                    for mc_i in range(MC):
                        for qt in range(NQT):
                            pj = ps_mm.tile([P, 512], F32, tag="mm")
                            for dc in range(DC):
                                nc.tensor.matmul(
                                    pj[:],
                                    w_sb[:, dc, mc_i * P : (mc_i + 1) * P],
                                    xT[:, dc, qt * TB : (qt + 1) * TB],
                                    start=(dc == 0),
                                    stop=(dc == DC - 1),
                                )
                            nc.scalar.activation(
                                out_t[:, mc_i, qt * TB : (qt + 1) * TB],
                                pj[:],
                                Ident,
                                bias=b_sb[:, mc_i : mc_i + 1],
                            )
                for tc_i in range(NTC):
                    pv = ps_mm.tile([P, G], F32, tag="mm")
                    for dc in range(DC):
                        nc.tensor.matmul(
                            pv[:],
                            xT[:, dc, tc_i * P : (tc_i + 1) * P],
                            wv_sb[:, dc, :],
                            start=(dc == 0),
                            stop=(dc == DC - 1),
                        )
                    v_dst = v_sb[:].rearrange("p t (h c) -> p t h c", c=HD + 1)[
                        :, tc_i, :, 0:HD
                    ]
                    nc.vector.tensor_tensor(
                        v_dst,
                        pv[:].rearrange("p (h c) -> p h c", c=HD),
                        bv_bc[:].rearrange("p (h c) -> p h c", c=HD),
                        ADD,
                    )

            if st == 1:  # proj debug out
                with tc.tile_pool(name="dbg", bufs=2) as dbg:
                    for tc_i in range(TB // P):
                        d_sb = dbg.tile([P, D], F32, tag="dbg")
                        nc.vector.tensor_copy(
                            d_sb[:, 0:512], qT[:, 0, 0:512].bitcast(F32)
                        )
                        nc.vector.tensor_copy(
                            d_sb[:, 512:768], kT[:, 0, 0:256].bitcast(F32)
                        )
                        nc.vector.tensor_copy(
                            d_sb[:, 768:1024],
                            v_sb[:].rearrange("p t c -> p (t c)")[:, 0:256].bitcast(
                                F32
                            ),
                        )
                        nc.sync.dma_start(
                            y_d[tc_i * P : (tc_i + 1) * P, :], d_sb[:]
                        )

            if st >= 2:
                # ---------- phase 3: attention ----------
                work = tc.alloc_tile_pool(name="att", bufs=1)
                pTp = tc.alloc_tile_pool(name="pTp", bufs=10)
                smallp = tc.alloc_tile_pool(name="smallp", bufs=2)
                ctxn = [
                    work.tile([HD, S], F32R, tag=f"ctxn{h}", name=f"ctxn{h}")
                    for h in range(NH_LOC)
                ]
                v_heads = v_sb[:].rearrange("p t (h c) -> p t h c", c=HD + 1)
                for pair in range(MC):
                    for qt in range(NQT):
                        nkc = 4 * qt + 4
                        c_ps = [
                            ps_ctx.tile([P, 512], F32, tag="ctx", name=f"cps{h01}")
                            for h01 in range(2)
                        ]
                        for kcb in range(0, nkc, 4):  # blocks of <=4 k-chunks
                            kcs = list(range(kcb, min(kcb + 4, nkc)))
                            s_tiles = {}
                            for kc in kcs:
                                j = kc - 4 * qt
                                coff = max(0, j) * P
                                for h01 in range(2):
                                    pb = h01 * HD
                                    s_ps = ps_mm.tile([P, 512], F32, tag="mm")
                                    nc.tensor.matmul(
                                        s_ps[:, coff:512],
                                        kT[pb : pb + HD, pair, kc * P : (kc + 1) * P],
                                        qT[
                                            pb : pb + HD,
                                            pair,
                                            qt * TB + coff : (qt + 1) * TB,
                                        ],
                                        start=True,
                                        stop=True,
                                    )
                                    s_tiles[(kc, h01)] = (s_ps, coff)
                            p_tiles = {}
                            for kc in kcs:
                                j = kc - 4 * qt
                                for h01 in range(2):
                                    s_ps, coff = s_tiles[(kc, h01)]
                                    pT = pTp.tile([P, 512], F32R, tag="pT")
                                    nc.scalar.activation(
                                        pT[:, coff:512],
                                        s_ps[:, coff:512],
                                        Exp,
                                        scale=0.125,
                                    )
                                    if j >= 0:
                                        nc.vector.tensor_tensor(
                                            pT[:, coff : coff + P],
                                            pT[:, coff : coff + P],
                                            tri_r[:],
                                            MULT,
                                        )
                                    p_tiles[(kc, h01)] = (pT, coff)
                            for kc in kcs:
                                for h01 in range(2):
                                    pT, coff = p_tiles[(kc, h01)]
                                    h = 2 * pair + h01
                                    nc.tensor.matmul(
                                        c_ps[h01][0 : HD + 1, coff:512],
                                        v_heads[:, kc, h, :],
                                        pT[:, coff:512],
                                        start=(kc == 0),
                                        stop=(kc == nkc - 1),
                                    )
                        for h01 in range(2):
                            h = 2 * pair + h01
                            den = smallp.tile([P, 512], F32R, tag="den")
                            nc.scalar.activation(
                                den[64:65, :],
                                c_ps[h01][64:65, :],
                                mybir.ActivationFunctionType.Copy,
                            )
                            b_ps = ps_bc.tile([P, 512], F32, tag="bc")
                            nc.tensor.matmul(
                                b_ps[0:HD, :],
                                ones_r[64:65, 0:HD],
                                den[64:65, :],
                                start=True,
                                stop=True,
                            )
                            bb = smallp.tile([HD, 512], F32, tag="bb")
                            nc.vector.reciprocal(bb[:], b_ps[0:HD, :])
                            nc.vector.tensor_tensor(
                                ctxn[h][:, qt * TB : (qt + 1) * TB],
                                c_ps[h01][0:HD, :],
                                bb[:],
                                MULT,
                            )

                    # A2A sends for this head pair (emitted inside pair loop)
                    if st >= 3:
                        po2 = pair * NC * P
                        for sh in range(NC):
                            jb = sh % 4
                            for h01 in range(2):
                                h = 2 * pair + h01
                                nc.sync.dma_start(
                                    a2a_in[
                                        po2 + sh * P + h01 * HD : po2
                                        + sh * P
                                        + (h01 + 1) * HD,
                                        :,
                                    ],
                                    ctxn[h][:, jb * TB : (jb + 1) * TB],
                                )
                        if coll:
                            nc.gpsimd.collective_compute(
                                "AllToAll",
                                mybir.AluOpType.bypass,
                                ins=[a2a_in[po2 : po2 + NC * P, :].opt()],
                                outs=[a2a_out[po2 : po2 + NC * P, :].opt()],
                                replica_groups=[list(range(NC))],
                            )

                if st == 2:  # attention debug out: raw ctxn tiles
                    for h in range(NH_LOC):
                        out_ap = (
                            y_d[h * P : (h + 1) * P, :]
                            .rearrange("a b -> (a b)")
                            .rearrange("(p t) -> p t", p=HD)
                        )
                        nc.sync.dma_start(out_ap, ctxn[h][:, :].bitcast(F32))

                nc.sync.dma_start(
                    wo_sb[:], wo_d.rearrange("(dc p) n -> p dc n", p=P)
                )
                smallp.release()
                pTp.release()
                work.release()

            if st >= 3:
                outp = tc.alloc_tile_pool(name="outp", bufs=1)
                ctxf = outp.tile([P, DC, TB], F32R, tag="ctxf")
                gsrc = a2a_out if coll else a2a_in
                if gather:
                    nc.gpsimd.dma_gather(
                        out_ap=ctxf[:],
                        in_ap=gsrc[:],
                        idxs_ap=gidx_sb[:],
                        num_idxs=D,
                        num_idxs_reg=D,
                        elem_size=TB,
                    )
                else:
                    for dc in range(DC):
                        nc.sync.dma_start(
                            ctxf[:, dc, :], gsrc[dc * P : (dc + 1) * P, :]
                        )

                if st == 3:  # a2a debug out: gathered ctxf cols 0:128 per dc
                    with tc.tile_pool(name="dbg3", bufs=2) as dbg3:
                        for tc_i in range(TB // P):
                            d_sb = dbg3.tile([P, D], F32, tag="dbg3")
                            for dc in range(DC):
                                nc.vector.tensor_copy(
                                    d_sb[:, dc * P : (dc + 1) * P],
                                    ctxf[:, dc, tc_i * P : (tc_i + 1) * P].bitcast(
                                        F32
                                    ),
                                )
                            nc.sync.dma_start(
                                y_d[tc_i * P : (tc_i + 1) * P, :], d_sb[:]
                            )

                if st >= 4:
                    # ---------- phase 5: output projection ----------
                    with tc.tile_pool(name="out_pool", bufs=3) as out_pool:
                        for tc_i in range(TB // P):
                            for nt in range(2):
                                po = ps_mm.tile([P, 512], F32, tag="mm")
                                for dc in range(DC):
                                    nc.tensor.matmul(
                                        po[:],
                                        ctxf[:, dc, tc_i * P : (tc_i + 1) * P],
                                        wo_sb[:, dc, nt * 512 : (nt + 1) * 512],
                                        start=(dc == 0),
                                        stop=(dc == DC - 1),
                                    )
                                o_sb = out_pool.tile([P, 512], F32, tag="osb")
                                nc.vector.tensor_tensor(
                                    o_sb[:],
                                    po[:],
                                    bo_bc[:, nt * 512 : (nt + 1) * 512],
                                    ADD,
                                )
                                nc.sync.dma_start(
                                    y_d[
                                        tc_i * P : (tc_i + 1) * P,
                                        nt * 512 : (nt + 1) * 512,
                                    ],
                                    o_sb[:],
                                )

                outp.release()

    nc.compile()
    return nc


_NC_CACHE = {}


def _get_nc():
    if "nc" not in _NC_CACHE:
        _NC_CACHE["nc"] = build()
    return _NC_CACHE["nc"]


def _make_in_maps(x, Wq, bq, Wk, bk, Wv, bv, Wo, bo):
    x = np.asarray(x, np.float32)
    Wq, Wk, Wv, Wo = (np.asarray(a, np.float32) for a in (Wq, Wk, Wv, Wo))
    bq, bk, bv, bo = (np.asarray(a, np.float32) for a in (bq, bk, bv, bo))
    in_maps = []
    for c in range(NC):
        b, g = c // 4, c % 4
        sl = slice(g * G, (g + 1) * G)
        dd = np.arange(D)
        gp = (dd % G) // P  # head-pair within group
        gg = dd // G        # source group
        rr = dd % P
        gidx = (gp * (NC * P) + P * (4 * b + gg) + rr).astype(np.int16)
        in_maps.append(
            {
                "x": np.ascontiguousarray(x[b]),
                "wq": np.ascontiguousarray(Wq[:, sl]),
                "wk": np.ascontiguousarray(Wk[:, sl]),
                "wv": np.ascontiguousarray(Wv[:, sl]),
                "bq": np.ascontiguousarray(bq[sl].reshape(MC, P).T),
                "bk": np.ascontiguousarray(bk[sl].reshape(MC, P).T),
                "bv": np.ascontiguousarray(bv[sl].reshape(1, G)),
                "wo": Wo,
                "bo": np.ascontiguousarray(bo.reshape(1, D)),
                "gidx": np.ascontiguousarray(np.tile(gidx.reshape(D // 16, 16).T, (8, 1))),
            }
        )
    return in_maps


def run(inputs, trace=False, tmpdir=None):
    """Run on 8 cores; returns (output [2,2048,1024], BassKernelResults)."""
    if trace:
        _install_ntff_hook()
    nc = _get_nc()
    in_maps = _make_in_maps(**inputs)
    res = bass_utils.run_bass_kernel_spmd(
        nc, in_maps, core_ids=list(range(NC)), trace=trace, tmpdir=tmpdir
    )
    out = np.empty((B, S, D), np.float32)
    for c in range(NC):
        b, g = c // 4, c % 4
        out[b, g * TB : (g + 1) * TB, :] = res.results[c]["y"]
    return out, res


def kernel(**inputs) -> np.ndarray:
    out, _ = run(inputs, trace=False)
    return out


# revision 18
# speedup vs baseline: 1.0512x; 1.0512x over previous
"""GPT self-attention layer (B=2, S=2048, D=1024, H=16, hd=64) on 8 TRN2 cores.

Sharding: data-parallel over batch (2) x tensor-parallel over heads (4 groups
of 4 heads). Core c handles batch b=c//4, head group g=c%4.

Per-core pipeline (all matmuls in float32r, ~1.5e-4 relative rounding):
  1. Transpose x [2048,1024] -> xT [1024-part, 2048] via PE transposes.
  2. QT/KT = W.T @ x.T (+bias via ACT copy), V = x @ Wv (+bias via DVE add),
     V stored with an interleaved ones-column per head (softmax denominator).
  3. Attention per head, scoresT orientation [k-part, q-free]:
     scoresT = KT.T @ QT (row-tiled 64x128, two heads concurrently),
     pT = exp(0.125 * scoresT) on ACT, causal diag masked by DVE multiply,
     psum_c[65,512] += [V|1].T @ pT accumulated over k-chunks
     (row 64 = softmax denominator), then normalize:
     recip (DVE) -> K=1 broadcast matmul at partition base 64 -> DVE mult.
  4. AllToAll (8 cores, shards duplicated across batch halves so the program
     is core-independent); receive side uses dma_gather with per-core indices.
  5. out = ctxT_full.T @ Wo + bo (bo added via DVE using a partition-broadcast
     copy of bo), DMA to y [512, 1024] (this core's token block).

build(stage): stage in {"proj", "attn", "a2a", "full"} for bisection; partial
stages write debug data to y instead of the final output.
"""

import contextlib
import ctypes
import sys
import types

sys.path.insert(0, "/opt/trn_rl_repo")

import numpy as np

import concourse.bass as bass
import concourse.mybir as mybir
import concourse.tile as tile
from concourse import bacc
from concourse import bass_utils
from concourse.masks import make_identity

P = 128
B, S, D = 2, 2048, 1024
NH_LOC = 4          # heads per core
HD = 64             # head dim
G = NH_LOC * HD     # local head dims = 256
MC = G // P         # m-chunks of local dims = 2
DC = D // P         # d-chunks = 8
TB = 512            # token block (output tokens per core, q-tile width)
NQT = S // TB       # q-tiles = 4
NTC = S // P        # token chunks = 16
NC = 8

F32 = mybir.dt.float32
F32R = mybir.dt.float32r
I16 = mybir.dt.int16
Exp = mybir.ActivationFunctionType.Exp
Ident = mybir.ActivationFunctionType.Identity
MULT = mybir.AluOpType.mult
ADD = mybir.AluOpType.add

_STAGES = {"proj": 1, "attn": 2, "a2a": 3, "full": 4}


def _install_ntff_hook():
    """Make trace=True work under axon: inject antenv.axon_hooks backed by
    ctypes calls into libaxon_pjrt.so (mirrors trn_agent_boot logic)."""
    if "antenv.axon_hooks" in sys.modules:
        return
    holder = {}
    mod = types.ModuleType("antenv.axon_hooks")
    mod.set_axon_ntff_profile_hook = lambda h: holder.update(h=h)
    mod.get_axon_ntff_profile_hook = lambda: holder.get("h")
    sys.modules["antenv.axon_hooks"] = mod
    try:
        lib = ctypes.CDLL("/opt/axon/libaxon_pjrt.so")
        if not hasattr(lib, "axon_start_nrt_profile"):
            return
    except OSError:
        return
    lib.axon_start_nrt_profile.argtypes = [
        ctypes.POINTER(ctypes.c_int64),
        ctypes.c_size_t,
    ]
    lib.axon_start_nrt_profile.restype = ctypes.c_int64
    lib.axon_stop_nrt_profile.argtypes = [ctypes.c_char_p]
    lib.axon_stop_nrt_profile.restype = ctypes.c_int64

    @contextlib.contextmanager
    def _hook(output_dir, device_ids):
        import jax

        jax.devices()
        if device_ids:
            ids = (ctypes.c_int64 * len(device_ids))(*device_ids)
            rc = lib.axon_start_nrt_profile(ids, len(device_ids))
        else:
            rc = lib.axon_start_nrt_profile(None, 0)
        if rc != 0:
            raise RuntimeError(f"axon_start_nrt_profile rc={rc}")
        try:
            yield
        finally:
            n = lib.axon_stop_nrt_profile(str(output_dir).encode())
            print(f"profile: {n} ntff file(s) written to {output_dir}")

    holder["h"] = _hook


def build(stage="full", coll=True, gather=True):
    st = _STAGES[stage]
    nc = bacc.Bacc("TRN2", target_bir_lowering=False, debug=False, num_devices=NC)

    x_d = nc.dram_tensor("x", [S, D], F32, kind="ExternalInput").ap()
    wq_d = nc.dram_tensor("wq", [D, G], F32R, kind="ExternalInput").ap()
    wk_d = nc.dram_tensor("wk", [D, G], F32R, kind="ExternalInput").ap()
    wv_d = nc.dram_tensor("wv", [D, G], F32R, kind="ExternalInput").ap()
    bq_d = nc.dram_tensor("bq", [P, MC], F32, kind="ExternalInput").ap()
    bk_d = nc.dram_tensor("bk", [P, MC], F32, kind="ExternalInput").ap()
    bv_d = nc.dram_tensor("bv", [1, G], F32, kind="ExternalInput").ap()
    wo_d = nc.dram_tensor("wo", [D, D], F32R, kind="ExternalInput").ap()
    bo_d = nc.dram_tensor("bo", [1, D], F32, kind="ExternalInput").ap()
    gidx_d = nc.dram_tensor("gidx", [P, D // 32], I16, kind="ExternalInput").ap()
    y_d = nc.dram_tensor("y", [TB, D], F32, kind="ExternalOutput").ap()

    with tile.TileContext(nc) as tc:
        with (
            tc.tile_pool(name="const", bufs=1) as const,
            tc.tile_pool(name="dram", bufs=1, space="DRAM") as dram,
            tc.tile_pool(name="ps_mm", bufs=4, space="PSUM") as ps_mm,
            tc.tile_pool(name="ps_ctx", bufs=2, space="PSUM") as ps_ctx,
            tc.tile_pool(name="ps_bc", bufs=2, space="PSUM") as ps_bc,
            tc.tile_pool(name="persist", bufs=1) as persist,
        ):
            # ---------------- constants ----------------
            ident = const.tile([P, P], F32, tag="ident")
            make_identity(nc, ident[:])
            ones_f = const.tile([P, 1], F32, tag="ones_f")
            nc.vector.memset(ones_f[:], 1.0)
            ones_r = const.tile([P, 65], F32R, tag="ones_r")
            nc.vector.tensor_copy(
                ones_r[:], ones_f[:, 0:1, None].to_broadcast((P, 65, 1))
            )
            # trimask[k, u] = 1 if k <= u else 0 (keep where u - k >= 0)
            tri_f = const.tile([P, P], F32, tag="tri_f")
            nc.gpsimd.memset(tri_f[:], 1.0)
            nc.gpsimd.affine_select(
                out=tri_f[:],
                in_=tri_f[:],
                compare_op=mybir.AluOpType.is_ge,
                fill=0.0,
                base=0,
                pattern=[[1, P]],
                channel_multiplier=-1,
            )
            tri_r = const.tile([P, P], F32R, tag="tri_r")
            nc.vector.tensor_copy(tri_r[:], tri_f[:])

            bq_sb = const.tile([P, MC], F32, tag="bq")
            bk_sb = const.tile([P, MC], F32, tag="bk")
            nc.sync.dma_start(bq_sb[:], bq_d)
            nc.sync.dma_start(bk_sb[:], bk_d)
            bv_row = const.tile([1, G], F32, tag="bv_row")
            nc.sync.dma_start(bv_row[:], bv_d)
            bv_bc = const.tile([P, G], F32, tag="bv_bc")
            nc.gpsimd.partition_broadcast(bv_bc[:], bv_row[:])
            bo_row = const.tile([1, D], F32, tag="bo_row")
            nc.sync.dma_start(bo_row[:], bo_d)
            bo_bc = const.tile([P, D], F32, tag="bo_bc")
            nc.gpsimd.partition_broadcast(bo_bc[:], bo_row[:])
            gidx_sb = const.tile([P, D // 32], I16, tag="gidx")
            nc.sync.dma_start(gidx_sb[:], gidx_d)

            # persistent activations
            qT = persist.tile([P, MC, S], F32R, tag="qT")
            kT = persist.tile([P, MC, S], F32R, tag="kT")
            v_sb = persist.tile([P, NTC, NH_LOC * (HD + 1)], F32R, tag="v")
            wo_sb = persist.tile([P, DC, D], F32R, tag="wo")

            # ones columns of v (denominator trick): col 64 of each head block
            v_ones_ap = v_sb[:].rearrange("p t (h c) -> p t h c", c=HD + 1)[
                :, :, :, HD
            ]
            nc.vector.tensor_copy(
                v_ones_ap,
                ones_f[:, 0:1, None].to_broadcast((P, NTC, NH_LOC, 1)),
            )

            a2a_in = [
                dram.tile([NC * P, TB], F32R, name=f"a2ain{p}", tag=f"a2ain{p}")
                for p in range(2)
            ]
            a2a_out = [
                dram.tile([NC * P, TB], F32R, name=f"a2aout{p}", tag=f"a2aout{p}")
                for p in range(2)
            ]

            with (
                tc.tile_pool(name="xw", bufs=1) as xw,
                tc.tile_pool(name="xnat", bufs=2) as xnat,
            ):
                wq_sb = xw.tile([P, DC, G], F32R, tag="wq")
                wk_sb = xw.tile([P, DC, G], F32R, tag="wk")
                wv_sb = xw.tile([P, DC, G], F32R, tag="wv")

                xT = xw.tile([P, DC, S], F32R, tag="xT")

                # ---------- phase 1: load + transpose x ----------
                for tc_i in range(NTC):
                    x_nat = xnat.tile([P, D], F32, tag="xnat")
                    nc.sync.dma_start(x_nat[:], x_d[tc_i * P : (tc_i + 1) * P, :])
                    for dcb in range(2):  # blocks of 4 d-chunks
                        tr_ps = ps_mm.tile([P, 512], F32, tag="mm")
                        for i in range(4):
                            dc = dcb * 4 + i
                            nc.tensor.transpose(
                                tr_ps[:, i * P : (i + 1) * P],
                                x_nat[:, dc * P : (dc + 1) * P],
                                ident[:],
                            )
                        nc.vector.tensor_copy(
                            xT[:, dcb * 4 : dcb * 4 + 4, tc_i * P : (tc_i + 1) * P],
                            tr_ps[:].rearrange("p (i u) -> p i u", i=4),
                        )

                nc.sync.dma_start(wq_sb[:], wq_d.rearrange("(dc p) m -> p dc m", p=P))
                nc.sync.dma_start(wk_sb[:], wk_d.rearrange("(dc p) m -> p dc m", p=P))
                nc.sync.dma_start(wv_sb[:], wv_d.rearrange("(dc p) m -> p dc m", p=P))
                # ---------- phase 2: projections ----------
                for w_sb, b_sb, out_t in ((wq_sb, bq_sb, qT), (wk_sb, bk_sb, kT)):
                    for mc_i in range(MC):
                        for qt in range(NQT):
                            pj = ps_mm.tile([P, 512], F32, tag="mm")
                            for dc in range(DC):
                                nc.tensor.matmul(
                                    pj[:],
                                    w_sb[:, dc, mc_i * P : (mc_i + 1) * P],
                                    xT[:, dc, qt * TB : (qt + 1) * TB],
                                    start=(dc == 0),
                                    stop=(dc == DC - 1),
                                )
                            nc.scalar.activation(
                                out_t[:, mc_i, qt * TB : (qt + 1) * TB],
                                pj[:],
                                Ident,
                                bias=b_sb[:, mc_i : mc_i + 1],
                            )
                for tc_i in range(NTC):
                    pv = ps_mm.tile([P, G], F32, tag="mm")
                    for dc in range(DC):
                        nc.tensor.matmul(
                            pv[:],
                            xT[:, dc, tc_i * P : (tc_i + 1) * P],
                            wv_sb[:, dc, :],
                            start=(dc == 0),
                            stop=(dc == DC - 1),
                        )
                    v_dst = v_sb[:].rearrange("p t (h c) -> p t h c", c=HD + 1)[
                        :, tc_i, :, 0:HD
                    ]
                    nc.vector.tensor_tensor(
                        v_dst,
                        pv[:].rearrange("p (h c) -> p h c", c=HD),
                        bv_bc[:].rearrange("p (h c) -> p h c", c=HD),
                        ADD,
                    )

            if st == 1:  # proj debug out
                with tc.tile_pool(name="dbg", bufs=2) as dbg:
                    for tc_i in range(TB // P):
                        d_sb = dbg.tile([P, D], F32, tag="dbg")
                        nc.vector.tensor_copy(
                            d_sb[:, 0:512], qT[:, 0, 0:512].bitcast(F32)
                        )
                        nc.vector.tensor_copy(
                            d_sb[:, 512:768], kT[:, 0, 0:256].bitcast(F32)
                        )
                        nc.vector.tensor_copy(
                            d_sb[:, 768:1024],
                            v_sb[:].rearrange("p t c -> p (t c)")[:, 0:256].bitcast(
                                F32
                            ),
                        )
                        nc.sync.dma_start(
                            y_d[tc_i * P : (tc_i + 1) * P, :], d_sb[:]
                        )

            if st >= 2:
                # ---------- phase 3: attention ----------
                work = tc.alloc_tile_pool(name="att", bufs=1)
                pTp = tc.alloc_tile_pool(name="pTp", bufs=10)
                smallp = tc.alloc_tile_pool(name="smallp", bufs=2)
                ctxn = [
                    work.tile([HD, S], F32R, tag=f"ctxn{h}", name=f"ctxn{h}")
                    for h in range(NH_LOC)
                ]
                v_heads = v_sb[:].rearrange("p t (h c) -> p t h c", c=HD + 1)
                for pair in range(MC):
                    for qt in range(NQT):
                        nkc = 4 * qt + 4
                        c_ps = [
                            ps_ctx.tile([P, 512], F32, tag="ctx", name=f"cps{h01}")
                            for h01 in range(2)
                        ]
                        for kcb in range(0, nkc, 4):  # blocks of <=4 k-chunks
                            kcs = list(range(kcb, min(kcb + 4, nkc)))
                            s_tiles = {}
                            for kc in kcs:
                                j = kc - 4 * qt
                                coff = max(0, j) * P
                                for h01 in range(2):
                                    pb = h01 * HD
                                    s_ps = ps_mm.tile([P, 512], F32, tag="mm")
                                    nc.tensor.matmul(
                                        s_ps[:, coff:512],
                                        kT[pb : pb + HD, pair, kc * P : (kc + 1) * P],
                                        qT[
                                            pb : pb + HD,
                                            pair,
                                            qt * TB + coff : (qt + 1) * TB,
                                        ],
                                        start=True,
                                        stop=True,
                                    )
                                    s_tiles[(kc, h01)] = (s_ps, coff)
                            p_tiles = {}
                            for kc in kcs:
                                j = kc - 4 * qt
                                for h01 in range(2):
                                    s_ps, coff = s_tiles[(kc, h01)]
                                    pT = pTp.tile([P, 512], F32R, tag="pT")
                                    nc.scalar.activation(
                                        pT[:, coff:512],
                                        s_ps[:, coff:512],
                                        Exp,
                                        scale=0.125,
                                    )
                                    if j >= 0:
                                        nc.vector.tensor_tensor(
                                            pT[:, coff : coff + P],
                                            pT[:, coff : coff + P],
                                            tri_r[:],
                                            MULT,
                                        )
                                    p_tiles[(kc, h01)] = (pT, coff)
                            for kc in kcs:
                                for h01 in range(2):
                                    pT, coff = p_tiles[(kc, h01)]
                                    h = 2 * pair + h01
                                    nc.tensor.matmul(
                                        c_ps[h01][0 : HD + 1, coff:512],
                                        v_heads[:, kc, h, :],
                                        pT[:, coff:512],
                                        start=(kc == 0),
                                        stop=(kc == nkc - 1),
                                    )
                        for h01 in range(2):
                            h = 2 * pair + h01
                            den = smallp.tile([P, 512], F32R, tag="den")
                            nc.scalar.activation(
                                den[64:65, :],
                                c_ps[h01][64:65, :],
                                mybir.ActivationFunctionType.Copy,
                            )
                            b_ps = ps_bc.tile([P, 512], F32, tag="bc")
                            nc.tensor.matmul(
                                b_ps[0:HD, :],
                                ones_r[64:65, 0:HD],
                                den[64:65, :],
                                start=True,
                                stop=True,
                            )
                            bb = smallp.tile([HD, 512], F32, tag="bb")
                            nc.vector.reciprocal(bb[:], b_ps[0:HD, :])
                            nc.vector.tensor_tensor(
                                ctxn[h][:, qt * TB : (qt + 1) * TB],
                                c_ps[h01][0:HD, :],
                                bb[:],
                                MULT,
                            )

                    # A2A sends for this head pair (emitted inside pair loop)
                    if st >= 3:
                        for sh in range(NC):
                            jb = sh % 4
                            for h01 in range(2):
                                h = 2 * pair + h01
                                nc.sync.dma_start(
                                    a2a_in[pair][
                                        sh * P + h01 * HD : sh * P + (h01 + 1) * HD,
                                        :,
                                    ],
                                    ctxn[h][:, jb * TB : (jb + 1) * TB],
                                )
                        if coll:
                            nc.gpsimd.collective_compute(
                                "AllToAll",
                                mybir.AluOpType.bypass,
                                ins=[a2a_in[pair].opt()],
                                outs=[a2a_out[pair].opt()],
                                replica_groups=[list(range(NC))],
                            )

                if st == 2:  # attention debug out: raw ctxn tiles
                    for h in range(NH_LOC):
                        out_ap = (
                            y_d[h * P : (h + 1) * P, :]
                            .rearrange("a b -> (a b)")
                            .rearrange("(p t) -> p t", p=HD)
                        )
                        nc.sync.dma_start(out_ap, ctxn[h][:, :].bitcast(F32))

                nc.sync.dma_start(
                    wo_sb[:], wo_d.rearrange("(dc p) n -> p dc n", p=P)
                )
                smallp.release()
                pTp.release()
                work.release()

            if st >= 3:
                outp = tc.alloc_tile_pool(name="outp", bufs=1)
                # ctxf[p, pr, g, t]: global dim chunk dc = 2*g + pr
                ctxf = outp.tile([P, 2, NQT, TB], F32R, tag="ctxf")
                gsrc = a2a_out if coll else a2a_in
                for pr in range(2):
                    nc.gpsimd.dma_gather(
                        out_ap=ctxf[:, pr],
                        in_ap=gsrc[pr][:],
                        idxs_ap=gidx_sb[:],
                        num_idxs=D // 2,
                        num_idxs_reg=D // 2,
                        elem_size=TB,
                    )

                if st == 3:  # a2a debug out: gathered ctxf cols 0:128 per dc
                    with tc.tile_pool(name="dbg3", bufs=2) as dbg3:
                        for tc_i in range(TB // P):
                            d_sb = dbg3.tile([P, D], F32, tag="dbg3")
                            for dc in range(DC):
                                nc.vector.tensor_copy(
                                    d_sb[:, dc * P : (dc + 1) * P],
                                    ctxf[
                                        :, dc % 2, dc // 2, tc_i * P : (tc_i + 1) * P
                                    ].bitcast(F32),
                                )
                            nc.sync.dma_start(
                                y_d[tc_i * P : (tc_i + 1) * P, :], d_sb[:]
                            )

                if st >= 4:
                    # ---------- phase 5: output projection ----------
                    with tc.tile_pool(name="out_pool", bufs=3) as out_pool:
                        for tc_i in range(TB // P):
                            for nt in range(2):
                                po = ps_mm.tile([P, 512], F32, tag="mm")
                                for i, dcg in enumerate(
                                    [2 * g for g in range(NQT)]
                                    + [2 * g + 1 for g in range(NQT)]
                                ):
                                    pr, g = dcg % 2, dcg // 2
                                    nc.tensor.matmul(
                                        po[:],
                                        ctxf[:, pr, g, tc_i * P : (tc_i + 1) * P],
                                        wo_sb[:, dcg, nt * 512 : (nt + 1) * 512],
                                        start=(i == 0),
                                        stop=(i == DC - 1),
                                    )
                                o_sb = out_pool.tile([P, 512], F32, tag="osb")
                                nc.vector.tensor_tensor(
                                    o_sb[:],
                                    po[:],
                                    bo_bc[:, nt * 512 : (nt + 1) * 512],
                                    ADD,
                                )
                                nc.sync.dma_start(
                                    y_d[
                                        tc_i * P : (tc_i + 1) * P,
                                        nt * 512 : (nt + 1) * 512,
                                    ],
                                    o_sb[:],
                                )

                outp.release()

    nc.compile()
    return nc


_NC_CACHE = {}


def _get_nc():
    if "nc" not in _NC_CACHE:
        _NC_CACHE["nc"] = build()
    return _NC_CACHE["nc"]


def _make_in_maps(x, Wq, bq, Wk, bk, Wv, bv, Wo, bo):
    x = np.asarray(x, np.float32)
    Wq, Wk, Wv, Wo = (np.asarray(a, np.float32) for a in (Wq, Wk, Wv, Wo))
    bq, bk, bv, bo = (np.asarray(a, np.float32) for a in (bq, bk, bv, bo))
    in_maps = []
    for c in range(NC):
        b, g = c // 4, c % 4
        sl = slice(g * G, (g + 1) * G)
        gidx = (b * (D // 2) + np.arange(D // 2)).astype(np.int16)
        in_maps.append(
            {
                "x": np.ascontiguousarray(x[b]),
                "wq": np.ascontiguousarray(Wq[:, sl]),
                "wk": np.ascontiguousarray(Wk[:, sl]),
                "wv": np.ascontiguousarray(Wv[:, sl]),
                "bq": np.ascontiguousarray(bq[sl].reshape(MC, P).T),
                "bk": np.ascontiguousarray(bk[sl].reshape(MC, P).T),
                "bv": np.ascontiguousarray(bv[sl].reshape(1, G)),
                "wo": Wo,
                "bo": np.ascontiguousarray(bo.reshape(1, D)),
                "gidx": np.ascontiguousarray(np.tile(gidx.reshape(D // 32, 16).T, (8, 1))),
            }
        )
    return in_maps


def run(inputs, trace=False, tmpdir=None):
    """Run on 8 cores; returns (output [2,2048,1024], BassKernelResults)."""
    if trace:
        _install_ntff_hook()
    nc = _get_nc()
    in_maps = _make_in_maps(**inputs)
    res = bass_utils.run_bass_kernel_spmd(
        nc, in_maps, core_ids=list(range(NC)), trace=trace, tmpdir=tmpdir
    )
    out = np.empty((B, S, D), np.float32)
    for c in range(NC):
        b, g = c // 4, c % 4
        out[b, g * TB : (g + 1) * TB, :] = res.results[c]["y"]
    return out, res


def kernel(**inputs) -> np.ndarray:
    out, _ = run(inputs, trace=False)
    return out
